# revision 2
# baseline (speedup 1.0000x reference)
"""Trainium2 Bass kernel for nn_NeuralTMT (sparse attention / embedding MF).

Math (per period s in 0..3, batch b, target t in {pos, neg}):
  x[b,s,:]   = mean_l LI[s, baskets[s,b,l], :]            (basket mean-pool)
  d[s]       = x[b,s,:] . IL[s, t_id, :]                  (raw attention dots)
  w          = softmax_s(mask_zero(d / sqrt(64)))
  attn       = sum_s w[s] * d[s]      (second einsum of the reference reduces
                                       to reusing the raw dots — no extra
                                       k-contraction needed)
  mf         = UI[s, uid[b], :] . IU[t_id, :]
  out        = sig(alpha_s) * attn + (1 - sig(alpha_s)) * mf

Sharding: data-parallel over batch. B=16384 -> 2048 rows per core on 8
NeuronCores; embedding tables replicated per core.

Gathers use the hardware vector-indirect DMA (one row index per partition,
contiguous fetch per partition), which is the only gather primitive whose
walrus lowering is exact on this platform:
  - basket rows:   one instruction per (tile, period, basket-slot), 128 rows
                   each, accumulated into the mean-pool sum via the SDMA
                   inline-add (CCE) when enabled, else via DVE reduction.
  - targets:       IL and IU rows for the same item id are fetched together
                   from a host-concatenated [IL || IU] table (512B rows).
  - user rows:     UI laid out as [user, 4*64] so one 1KB fetch per user
                   covers all four periods.
"""

import os
import sys

import numpy as np

sys.path.insert(0, "/opt/trn_rl_repo")

# ---- problem constants (hardcoded per contract) ----
S = 4            # periods
V = 100001       # item vocab (incl padding row)
U = 100000       # user vocab
K = 64           # embedding dim
L = 50           # basket length
B = 16384        # batch
NCORES = 8
BC = B // NCORES  # 2048 rows per core
P = 128
NT = BC // P      # 16 partition-tiles per core
NEG_BIG = -(2.0 ** 32) + 1.0

# basket accumulation: "cce" = inline add during DMA, "dve" = gather columns
# then reduce on the vector engine
ACC_MODE = os.environ.get("KERNEL_ACC_MODE", "dve")

_PROG_CACHE = {}
LAST_RESULTS = None  # BassKernelResults of the most recent run (for test.py)


def _build_program(acc_mode: str):
    import concourse.bacc as bacc
    import concourse.bass as bass
    import concourse.mybir as mybir
    import concourse.tile as tile

    f32 = mybir.dt.float32
    i32 = mybir.dt.int32
    A = mybir.AluOpType
    X = mybir.AxisListType.X

    nc = bacc.Bacc(
        "TRN2",
        target_bir_lowering=False,
        debug=False,
        enable_asserts=False,
        num_devices=NCORES,
    )

    LIt = nc.dram_tensor("LI", [S * V, K], f32, kind="ExternalInput")
    CATt = nc.dram_tensor("CAT", [S * V, 2 * K], f32, kind="ExternalInput")
    UIt = nc.dram_tensor("UIre", [U, S * K], f32, kind="ExternalInput")
    BIXt = nc.dram_tensor("bidx", [BC, S * L], i32, kind="ExternalInput")
    TIXt = nc.dram_tensor("tidx", [BC, 9], i32, kind="ExternalInput")
    SCt = nc.dram_tensor("scoef", [P, 16], f32, kind="ExternalInput")
    OUTt = nc.dram_tensor("out", [BC, 8], f32, kind="ExternalOutput")

    with tile.TileContext(nc) as tc:
        with (
            tc.tile_pool(name="const", bufs=1) as cpool,
            tc.tile_pool(name="gath", bufs=4) as gpool,
            tc.tile_pool(name="work", bufs=3) as wpool,
        ):
            SC = cpool.tile([P, 16], f32)
            nc.sync.dma_start(out=SC[:], in_=SCt.ap()[:, :])

            for t in range(NT):
                b0 = t * P
                bix = gpool.tile([P, S * L], i32, tag="bix")
                nc.sync.dma_start(out=bix[:], in_=BIXt.ap()[b0 : b0 + P, :])
                tix = gpool.tile([P, 9], i32, tag="tix")
                nc.sync.dma_start(out=tix[:], in_=TIXt.ap()[b0 : b0 + P, :])

                # ---- basket gathers + mean-pool sum ----
                Xs = wpool.tile([P, S * K], f32, tag="X")
                for s in range(S):
                    if acc_mode == "cce":
                        for l in range(L):
                            nc.gpsimd.indirect_dma_start(
                                out=Xs[:, s * K : (s + 1) * K],
                                out_offset=None,
                                in_=LIt.ap(),
                                in_offset=bass.IndirectOffsetOnAxis(
                                    ap=bix[:, s * L + l : s * L + l + 1], axis=0
                                ),
                                compute_op=A.add if l > 0 else A.bypass,
                            )
                    else:
                        G = gpool.tile([P, L * K], f32, tag="G")
                        for l in range(L):
                            nc.gpsimd.indirect_dma_start(
                                out=G[:, l * K : (l + 1) * K],
                                out_offset=None,
                                in_=LIt.ap(),
                                in_offset=bass.IndirectOffsetOnAxis(
                                    ap=bix[:, s * L + l : s * L + l + 1], axis=0
                                ),
                            )
                        nc.vector.reduce_sum(
                            out=Xs[:, s * K : (s + 1) * K],
                            in_=G[:].rearrange("p (l k) -> p k l", l=L, k=K),
                            axis=X,
                        )

                # ---- target gathers: TQ[:, j*128:(j+1)*128] = IL[j] || IU[j] ----
                TQ = gpool.tile([P, 8 * 2 * K], f32, tag="TQ")
                for j in range(8):
                    nc.gpsimd.indirect_dma_start(
                        out=TQ[:, j * 2 * K : (j + 1) * 2 * K],
                        out_offset=None,
                        in_=CATt.ap(),
                        in_offset=bass.IndirectOffsetOnAxis(
                            ap=tix[:, j : j + 1], axis=0
                        ),
                    )
                U4 = gpool.tile([P, S * K], f32, tag="U4")
                nc.gpsimd.indirect_dma_start(
                    out=U4[:],
                    out_offset=None,
                    in_=UIt.ap(),
                    in_offset=bass.IndirectOffsetOnAxis(ap=tix[:, 8:9], axis=0),
                )

                T8 = TQ[:].rearrange("p (j q) -> p j q", j=8, q=2 * K)[:, :, 0:K]
                Q8 = TQ[:].rearrange("p (j q) -> p j q", j=8, q=2 * K)[:, :, K : 2 * K]

                # ---- attention dots: dp[p, (j, s)] = X[s] . T8[j] ----
                prod = wpool.tile([P, 8 * S * K], f32, tag="prod")
                nc.vector.tensor_tensor(
                    out=prod[:].rearrange("p (j s k) -> p j s k", j=8, s=S, k=K),
                    in0=T8.unsqueeze(2).broadcast_to([P, 8, S, K]),
                    in1=Xs[:]
                    .rearrange("p (s k) -> p s k", s=S, k=K)
                    .unsqueeze(1)
                    .broadcast_to([P, 8, S, K]),
                    op=A.mult,
                )
                dp = wpool.tile([P, 32], f32, tag="dp")
                nc.vector.reduce_sum(
                    out=dp[:],
                    in_=prod[:].rearrange("p (j s k) -> p j s k", j=8, s=S, k=K),
                    axis=X,
                )

                # ---- masked softmax over s, reusing dp as the value vector ----
                mask = wpool.tile([P, 32], f32, tag="mask")
                nc.vector.tensor_scalar(
                    out=mask[:],
                    in0=dp[:],
                    scalar1=0.0,
                    scalar2=NEG_BIG,
                    op0=A.is_equal,
                    op1=A.mult,
                )
                dmm = wpool.tile([P, 32], f32, tag="dmm")
                # dp is 50*8=400x the reference's softmax logits
                nc.vector.tensor_scalar_mul(dmm[:], dp[:], 1.0 / 400.0)
                nc.vector.tensor_tensor(out=dmm[:], in0=dmm[:], in1=mask[:], op=A.add)
                mx = wpool.tile([P, 8], f32, tag="mx")
                nc.vector.reduce_max(
                    out=mx[:],
                    in_=dmm[:].rearrange("p (j s) -> p j s", j=8, s=S),
                    axis=X,
                )
                dsub = wpool.tile([P, 32], f32, tag="dsub")
                nc.vector.tensor_tensor(
                    out=dsub[:].rearrange("p (j s) -> p j s", j=8, s=S),
                    in0=dmm[:].rearrange("p (j s) -> p j s", j=8, s=S),
                    in1=mx[:].unsqueeze(2).broadcast_to([P, 8, S]),
                    op=A.subtract,
                )
                e = wpool.tile([P, 32], f32, tag="e")
                nc.scalar.activation(
                    out=e[:], in_=dsub[:], func=mybir.ActivationFunctionType.Exp
                )
                den = wpool.tile([P, 8], f32, tag="den")
                nc.vector.reduce_sum(
                    out=den[:], in_=e[:].rearrange("p (j s) -> p j s", j=8, s=S), axis=X
                )
                fnum = wpool.tile([P, 32], f32, tag="fnum")
                nc.vector.tensor_tensor(out=fnum[:], in0=e[:], in1=dp[:], op=A.mult)
                num = wpool.tile([P, 8], f32, tag="num")
                nc.vector.reduce_sum(
                    out=num[:],
                    in_=fnum[:].rearrange("p (j s) -> p j s", j=8, s=S),
                    axis=X,
                )
                rden = wpool.tile([P, 8], f32, tag="rden")
                nc.vector.reciprocal(rden[:], den[:])
                attn = wpool.tile([P, 8], f32, tag="attn")
                nc.vector.tensor_tensor(out=attn[:], in0=num[:], in1=rden[:], op=A.mult)

                # ---- MF dots: mf[p, (per, pn)] = U4[per] . Q8[(per, pn)] ----
                prod2 = wpool.tile([P, 8 * K], f32, tag="prod2")
                nc.vector.tensor_tensor(
                    out=prod2[:].rearrange(
                        "p (per pn k) -> p per pn k", per=S, pn=2, k=K
                    ),
                    in0=Q8.rearrange("p (per pn) q -> p per pn q", per=S, pn=2),
                    in1=U4[:]
                    .rearrange("p (per k) -> p per k", per=S, k=K)
                    .unsqueeze(2)
                    .broadcast_to([P, S, 2, K]),
                    op=A.mult,
                )
                mf = wpool.tile([P, 8], f32, tag="mf")
                nc.vector.reduce_sum(
                    out=mf[:],
                    in_=prod2[:].rearrange(
                        "p (per pn k) -> p per pn k", per=S, pn=2, k=K
                    ),
                    axis=X,
                )

                # ---- combine: out = (sig/50)*attn' + (1-sig)*mf ----
                o1 = wpool.tile([P, 8], f32, tag="o1")
                nc.vector.tensor_tensor(out=o1[:], in0=attn[:], in1=SC[:, 0:8], op=A.mult)
                o2 = wpool.tile([P, 8], f32, tag="o2")
                nc.vector.tensor_tensor(out=o2[:], in0=mf[:], in1=SC[:, 8:16], op=A.mult)
                fin = wpool.tile([P, 8], f32, tag="fin")
                nc.vector.tensor_tensor(out=fin[:], in0=o1[:], in1=o2[:], op=A.add)

                nc.sync.dma_start(out=OUTt.ap()[b0 : b0 + P, :], in_=fin[:])

    nc.compile()
    return nc


def _get_program(acc_mode: str):
    if acc_mode not in _PROG_CACHE:
        _PROG_CACHE[acc_mode] = _build_program(acc_mode)
    return _PROG_CACHE[acc_mode]


def _prep_inputs(IL, LI, UI, IU, alpha, uid, baskets, iid, neg_iid):
    """Host-side input prep: flatten tables, fold period offsets into indices."""
    LIflat = np.ascontiguousarray(LI.reshape(S * V, K)).astype(np.float32, copy=False)
    ILflat = IL.reshape(S * V, K)
    CAT = np.empty((S * V, 2 * K), np.float32)
    CAT[:, 0:K] = ILflat
    CAT[:, K : 2 * K] = np.broadcast_to(IU[None, :, :], (S, V, K)).reshape(S * V, K)
    UIre = np.ascontiguousarray(UI.transpose(1, 0, 2).reshape(U, S * K)).astype(
        np.float32, copy=False
    )

    per_off_V = (np.arange(S, dtype=np.int64) * V)[:, None]  # [S,1]
    bidx = (
        (baskets.astype(np.int64) + per_off_V[:, :, None])
        .transpose(1, 0, 2)
        .reshape(B, S * L)
        .astype(np.int32)
    )
    tidx = np.empty((B, 9), np.int32)
    tpos = (iid.astype(np.int64) + per_off_V).T  # [B,S]
    tneg = (neg_iid.astype(np.int64) + per_off_V).T
    tidx[:, 0:8:2] = tpos
    tidx[:, 1:8:2] = tneg
    tidx[:, 8] = uid.astype(np.int64)

    sig = 1.0 / (1.0 + np.exp(-alpha.astype(np.float64)))  # [S]
    scoef = np.empty((P, 16), np.float32)
    jper = np.arange(8) // 2
    scoef[:, 0:8] = (sig[jper] / L).astype(np.float32)[None, :]
    scoef[:, 8:16] = (1.0 - sig[jper]).astype(np.float32)[None, :]

    in_maps = []
    for c in range(NCORES):
        in_maps.append(
            dict(
                LI=LIflat,
                CAT=CAT,
                UIre=UIre,
                bidx=bidx[c * BC : (c + 1) * BC],
                tidx=tidx[c * BC : (c + 1) * BC],
                scoef=scoef,
            )
        )
    return in_maps


def kernel(IL, LI, UI, IU, alpha, uid, baskets, iid, neg_iid):
    global LAST_RESULTS
    from concourse import bass_utils

    nc = _get_program(ACC_MODE)
    in_maps = _prep_inputs(IL, LI, UI, IU, alpha, uid, baskets, iid, neg_iid)
    trace = os.environ.get("KERNEL_TRACE", "0") == "1"
    res = bass_utils.run_bass_kernel_spmd(
        nc, in_maps, core_ids=list(range(NCORES)), trace=trace
    )
    LAST_RESULTS = res
    outs = np.concatenate([r["out"] for r in res.results], axis=0)  # [B, 8]
    return tuple(np.ascontiguousarray(outs[:, j]) for j in range(8))


# revision 4
# speedup vs baseline: 1.0004x; 1.0004x over previous
"""Trainium2 Bass kernel for nn_NeuralTMT (sparse attention / embedding MF).

Math (per period s in 0..3, batch b, target t in {pos, neg}):
  x[b,s,:]   = mean_l LI[s, baskets[s,b,l], :]            (basket mean-pool)
  d[s]       = x[b,s,:] . IL[s, t_id, :]                  (raw attention dots)
  w          = softmax_s(mask_zero(d / sqrt(64)))
  attn       = sum_s w[s] * d[s]      (second einsum of the reference reduces
                                       to reusing the raw dots — no extra
                                       k-contraction needed)
  mf         = UI[s, uid[b], :] . IU[t_id, :]
  out        = sig(alpha_s) * attn + (1 - sig(alpha_s)) * mf

Sharding: data-parallel over batch. B=16384 -> 2048 rows per core on 8
NeuronCores; embedding tables replicated per core.

Gathers use the hardware vector-indirect DMA (one row index per partition,
contiguous fetch per partition), which is the only gather primitive whose
walrus lowering is exact on this platform:
  - basket rows:   one instruction per (tile, period, basket-slot), 128 rows
                   each, accumulated into the mean-pool sum via the SDMA
                   inline-add (CCE) when enabled, else via DVE reduction.
  - targets:       IL and IU rows for the same item id are fetched together
                   from a host-concatenated [IL || IU] table (512B rows).
  - user rows:     UI laid out as [user, 4*64] so one 1KB fetch per user
                   covers all four periods.
"""

import os
import sys

import numpy as np

sys.path.insert(0, "/opt/trn_rl_repo")

# ---- problem constants (hardcoded per contract) ----
S = 4            # periods
V = 100001       # item vocab (incl padding row)
U = 100000       # user vocab
K = 64           # embedding dim
L = 50           # basket length
B = 16384        # batch
NCORES = 8
BC = B // NCORES  # 2048 rows per core
P = 128
NT = BC // P      # 16 partition-tiles per core
NEG_BIG = -(2.0 ** 32) + 1.0

# basket accumulation: "cce" = inline add during DMA, "dve" = gather columns
# then reduce on the vector engine
ACC_MODE = os.environ.get("KERNEL_ACC_MODE", "dve")

_PROG_CACHE = {}
LAST_RESULTS = None  # BassKernelResults of the most recent run (for test.py)


def _build_program(acc_mode: str):
    import concourse.bacc as bacc
    import concourse.bass as bass
    import concourse.mybir as mybir
    import concourse.tile as tile

    f32 = mybir.dt.float32
    i32 = mybir.dt.int32
    A = mybir.AluOpType
    X = mybir.AxisListType.X

    nc = bacc.Bacc(
        "TRN2",
        target_bir_lowering=False,
        debug=False,
        enable_asserts=False,
        num_devices=NCORES,
    )

    LIt = nc.dram_tensor("LI", [S * V, K], f32, kind="ExternalInput")
    CATt = nc.dram_tensor("CAT", [S * V, 2 * K], f32, kind="ExternalInput")
    UIt = nc.dram_tensor("UIre", [U, S * K], f32, kind="ExternalInput")
    BIXt = nc.dram_tensor("bidx", [BC, S * L], i32, kind="ExternalInput")
    TIXt = nc.dram_tensor("tidx", [BC, 9], i32, kind="ExternalInput")
    SCt = nc.dram_tensor("scoef", [P, 16], f32, kind="ExternalInput")
    OUTt = nc.dram_tensor("out", [BC, 8], f32, kind="ExternalOutput")

    with tile.TileContext(nc) as tc:
        with (
            tc.tile_pool(name="const", bufs=1) as cpool,
            tc.tile_pool(name="gbig", bufs=6) as gbig,
            tc.tile_pool(name="gath", bufs=4) as gpool,
            tc.tile_pool(name="work", bufs=3) as wpool,
        ):
            SC = cpool.tile([P, 16], f32)
            nc.sync.dma_start(out=SC[:], in_=SCt.ap()[:, :])

            for t in range(NT):
                b0 = t * P
                bix = gpool.tile([P, S * L], i32, tag="bix")
                nc.sync.dma_start(out=bix[:], in_=BIXt.ap()[b0 : b0 + P, :])
                tix = gpool.tile([P, 9], i32, tag="tix")
                nc.sync.dma_start(out=tix[:], in_=TIXt.ap()[b0 : b0 + P, :])

                # ---- basket gathers + mean-pool sum ----
                Xs = wpool.tile([P, S * K], f32, tag="X")
                for s in range(S):
                    if acc_mode == "cce":
                        for l in range(L):
                            nc.gpsimd.indirect_dma_start(
                                out=Xs[:, s * K : (s + 1) * K],
                                out_offset=None,
                                in_=LIt.ap(),
                                in_offset=bass.IndirectOffsetOnAxis(
                                    ap=bix[:, s * L + l : s * L + l + 1], axis=0
                                ),
                                compute_op=A.add if l > 0 else A.bypass,
                            )
                    else:
                        G = gbig.tile([P, L * K], f32, tag="G")
                        for l in range(L):
                            nc.gpsimd.indirect_dma_start(
                                out=G[:, l * K : (l + 1) * K],
                                out_offset=None,
                                in_=LIt.ap(),
                                in_offset=bass.IndirectOffsetOnAxis(
                                    ap=bix[:, s * L + l : s * L + l + 1], axis=0
                                ),
                            )
                        nc.vector.reduce_sum(
                            out=Xs[:, s * K : (s + 1) * K],
                            in_=G[:].rearrange("p (l k) -> p k l", l=L, k=K),
                            axis=X,
                        )

                # ---- target gathers: TQ[:, j*128:(j+1)*128] = IL[j] || IU[j] ----
                TQ = gpool.tile([P, 8 * 2 * K], f32, tag="TQ")
                for j in range(8):
                    nc.gpsimd.indirect_dma_start(
                        out=TQ[:, j * 2 * K : (j + 1) * 2 * K],
                        out_offset=None,
                        in_=CATt.ap(),
                        in_offset=bass.IndirectOffsetOnAxis(
                            ap=tix[:, j : j + 1], axis=0
                        ),
                    )
                U4 = gpool.tile([P, S * K], f32, tag="U4")
                nc.gpsimd.indirect_dma_start(
                    out=U4[:],
                    out_offset=None,
                    in_=UIt.ap(),
                    in_offset=bass.IndirectOffsetOnAxis(ap=tix[:, 8:9], axis=0),
                )

                T8 = TQ[:].rearrange("p (j q) -> p j q", j=8, q=2 * K)[:, :, 0:K]
                Q8 = TQ[:].rearrange("p (j q) -> p j q", j=8, q=2 * K)[:, :, K : 2 * K]

                # ---- attention dots: dp[p, (j, s)] = X[s] . T8[j] ----
                prod = wpool.tile([P, 8 * S * K], f32, tag="prod")
                nc.vector.tensor_tensor(
                    out=prod[:].rearrange("p (j s k) -> p j s k", j=8, s=S, k=K),
                    in0=T8.unsqueeze(2).broadcast_to([P, 8, S, K]),
                    in1=Xs[:]
                    .rearrange("p (s k) -> p s k", s=S, k=K)
                    .unsqueeze(1)
                    .broadcast_to([P, 8, S, K]),
                    op=A.mult,
                )
                dp = wpool.tile([P, 32], f32, tag="dp")
                nc.vector.reduce_sum(
                    out=dp[:],
                    in_=prod[:].rearrange("p (j s k) -> p j s k", j=8, s=S, k=K),
                    axis=X,
                )

                # ---- masked softmax over s, reusing dp as the value vector ----
                mask = wpool.tile([P, 32], f32, tag="mask")
                nc.vector.tensor_scalar(
                    out=mask[:],
                    in0=dp[:],
                    scalar1=0.0,
                    scalar2=NEG_BIG,
                    op0=A.is_equal,
                    op1=A.mult,
                )
                dmm = wpool.tile([P, 32], f32, tag="dmm")
                # dp is 50*8=400x the reference's softmax logits
                nc.vector.tensor_scalar_mul(dmm[:], dp[:], 1.0 / 400.0)
                nc.vector.tensor_tensor(out=dmm[:], in0=dmm[:], in1=mask[:], op=A.add)
                mx = wpool.tile([P, 8], f32, tag="mx")
                nc.vector.reduce_max(
                    out=mx[:],
                    in_=dmm[:].rearrange("p (j s) -> p j s", j=8, s=S),
                    axis=X,
                )
                dsub = wpool.tile([P, 32], f32, tag="dsub")
                nc.vector.tensor_tensor(
                    out=dsub[:].rearrange("p (j s) -> p j s", j=8, s=S),
                    in0=dmm[:].rearrange("p (j s) -> p j s", j=8, s=S),
                    in1=mx[:].unsqueeze(2).broadcast_to([P, 8, S]),
                    op=A.subtract,
                )
                e = wpool.tile([P, 32], f32, tag="e")
                nc.scalar.activation(
                    out=e[:], in_=dsub[:], func=mybir.ActivationFunctionType.Exp
                )
                den = wpool.tile([P, 8], f32, tag="den")
                nc.vector.reduce_sum(
                    out=den[:], in_=e[:].rearrange("p (j s) -> p j s", j=8, s=S), axis=X
                )
                fnum = wpool.tile([P, 32], f32, tag="fnum")
                nc.vector.tensor_tensor(out=fnum[:], in0=e[:], in1=dp[:], op=A.mult)
                num = wpool.tile([P, 8], f32, tag="num")
                nc.vector.reduce_sum(
                    out=num[:],
                    in_=fnum[:].rearrange("p (j s) -> p j s", j=8, s=S),
                    axis=X,
                )
                rden = wpool.tile([P, 8], f32, tag="rden")
                nc.vector.reciprocal(rden[:], den[:])
                attn = wpool.tile([P, 8], f32, tag="attn")
                nc.vector.tensor_tensor(out=attn[:], in0=num[:], in1=rden[:], op=A.mult)

                # ---- MF dots: mf[p, (per, pn)] = U4[per] . Q8[(per, pn)] ----
                prod2 = wpool.tile([P, 8 * K], f32, tag="prod2")
                nc.vector.tensor_tensor(
                    out=prod2[:].rearrange(
                        "p (per pn k) -> p per pn k", per=S, pn=2, k=K
                    ),
                    in0=Q8.rearrange("p (per pn) q -> p per pn q", per=S, pn=2),
                    in1=U4[:]
                    .rearrange("p (per k) -> p per k", per=S, k=K)
                    .unsqueeze(2)
                    .broadcast_to([P, S, 2, K]),
                    op=A.mult,
                )
                mf = wpool.tile([P, 8], f32, tag="mf")
                nc.vector.reduce_sum(
                    out=mf[:],
                    in_=prod2[:].rearrange(
                        "p (per pn k) -> p per pn k", per=S, pn=2, k=K
                    ),
                    axis=X,
                )

                # ---- combine: out = (sig/50)*attn' + (1-sig)*mf ----
                o1 = wpool.tile([P, 8], f32, tag="o1")
                nc.vector.tensor_tensor(out=o1[:], in0=attn[:], in1=SC[:, 0:8], op=A.mult)
                o2 = wpool.tile([P, 8], f32, tag="o2")
                nc.vector.tensor_tensor(out=o2[:], in0=mf[:], in1=SC[:, 8:16], op=A.mult)
                fin = wpool.tile([P, 8], f32, tag="fin")
                nc.vector.tensor_tensor(out=fin[:], in0=o1[:], in1=o2[:], op=A.add)

                nc.sync.dma_start(out=OUTt.ap()[b0 : b0 + P, :], in_=fin[:])

    nc.compile()
    return nc


def _get_program(acc_mode: str):
    if acc_mode not in _PROG_CACHE:
        _PROG_CACHE[acc_mode] = _build_program(acc_mode)
    return _PROG_CACHE[acc_mode]


def _prep_inputs(IL, LI, UI, IU, alpha, uid, baskets, iid, neg_iid):
    """Host-side input prep: flatten tables, fold period offsets into indices."""
    LIflat = np.ascontiguousarray(LI.reshape(S * V, K)).astype(np.float32, copy=False)
    ILflat = IL.reshape(S * V, K)
    CAT = np.empty((S * V, 2 * K), np.float32)
    CAT[:, 0:K] = ILflat
    CAT[:, K : 2 * K] = np.broadcast_to(IU[None, :, :], (S, V, K)).reshape(S * V, K)
    UIre = np.ascontiguousarray(UI.transpose(1, 0, 2).reshape(U, S * K)).astype(
        np.float32, copy=False
    )

    per_off_V = (np.arange(S, dtype=np.int64) * V)[:, None]  # [S,1]
    bidx = (
        (baskets.astype(np.int64) + per_off_V[:, :, None])
        .transpose(1, 0, 2)
        .reshape(B, S * L)
        .astype(np.int32)
    )
    tidx = np.empty((B, 9), np.int32)
    tpos = (iid.astype(np.int64) + per_off_V).T  # [B,S]
    tneg = (neg_iid.astype(np.int64) + per_off_V).T
    tidx[:, 0:8:2] = tpos
    tidx[:, 1:8:2] = tneg
    tidx[:, 8] = uid.astype(np.int64)

    sig = 1.0 / (1.0 + np.exp(-alpha.astype(np.float64)))  # [S]
    scoef = np.empty((P, 16), np.float32)
    jper = np.arange(8) // 2
    scoef[:, 0:8] = (sig[jper] / L).astype(np.float32)[None, :]
    scoef[:, 8:16] = (1.0 - sig[jper]).astype(np.float32)[None, :]

    in_maps = []
    for c in range(NCORES):
        in_maps.append(
            dict(
                LI=LIflat,
                CAT=CAT,
                UIre=UIre,
                bidx=bidx[c * BC : (c + 1) * BC],
                tidx=tidx[c * BC : (c + 1) * BC],
                scoef=scoef,
            )
        )
    return in_maps


def kernel(IL, LI, UI, IU, alpha, uid, baskets, iid, neg_iid):
    global LAST_RESULTS
    from concourse import bass_utils

    nc = _get_program(ACC_MODE)
    in_maps = _prep_inputs(IL, LI, UI, IU, alpha, uid, baskets, iid, neg_iid)
    trace = os.environ.get("KERNEL_TRACE", "0") == "1"
    res = bass_utils.run_bass_kernel_spmd(
        nc, in_maps, core_ids=list(range(NCORES)), trace=trace
    )
    LAST_RESULTS = res
    outs = np.concatenate([r["out"] for r in res.results], axis=0)  # [B, 8]
    return tuple(np.ascontiguousarray(outs[:, j]) for j in range(8))
